# revision 1
# baseline (speedup 1.0000x reference)
"""CapsuleLayer dynamic-routing kernel for 8 Trainium2 NeuronCores.

Problem: u_hat[b,i,j,e] = einsum('bid,ijde->bije', x, W) with
B=64, I=2304, D=8, J=32, E=16, followed by NUM_ROUTING=3 softmax
routing iterations.  Output V = squash(S_2) with shape [B, J, E].

Strategy (data-parallel over batch, 8 b per core):
 - Host pre-lays W into fp16 tiles [G=144, 128, 512] with partition
   p = (i_loc*8 + d) and free f = (e*32 + j); W streams through SBUF once.
 - A block-diagonal lhsT (built on-chip from x/J with a mask multiply)
   makes ONE matmul produce u_hat for 16 i's x 8 b x (all j,e) per group;
   a second matmul per group accumulates S_0 = (1/J) sum_i u_hat in PSUM.
 - u_hat stays resident in SBUF as fp16 [128=(il,b), G*512=(g,(e,j))] —
   it never round-trips to HBM.
 - Each routing iteration (DVE TensorTensor ops run in 2x mode for packed
   fp16): P = u_hat*V_rep, in-place e-reduction tree for the agreement
   logits, one batched ACT exp for softmax, T = u_hat*c into the dead P
   tile, and PE contracts over i with a constant indicator lhsT,
   accumulating S in PSUM.  Two-stage software pipelining plus triple
   buffering of the product tile keeps DVE (the bottleneck) saturated.
"""

import sys

import numpy as np

sys.path.insert(0, "/opt/trn_rl_repo")

B, I, D, J, E = 64, 2304, 8, 32, 16
NC_CORES = 8
BS = B // NC_CORES          # 8 batch elements per core
IL = 16                     # i's per group
G = I // IL                 # 144 groups
F = J * E                   # 512 free elements per group
GB = 6                      # groups per batched DVE macro-op
P_BUFS = 4                  # product-tile buffering
W_BUFS = 3                  # W-stream buffering
SM_BUFS = 2                 # softmax small-tile buffering
GP_EVERY = 0                # offload P-mult of every Nth macro to GPSIMD (0=off)
NUM_ROUTING = 3

_CACHE = {}


def _build_program(n_groups, nonzero_b0, n_passes=2, n_bodies=1):
    import concourse.bass as bass
    import concourse.mybir as mybir
    import concourse.tile as tile
    from concourse import bacc

    fp16 = mybir.dt.float16
    f32 = mybir.dt.float32

    nc = bacc.Bacc("TRN2", target_bir_lowering=False, debug=False)

    # register the squash-epsilon constant for activation bias
    eps_t = nc.alloc_sbuf_tensor("const-f32-eps", [128, 1], f32)
    nc.gpsimd.memset(eps_t.ap(), 1e-7)
    nc.const_aps.aps[(f32, 1e-7)] = eps_t.ap()
    nc.all_engine_barrier()

    g_ = n_groups
    wp = nc.dram_tensor("wp", [g_, 128, F], fp16, kind="ExternalInput").ap()
    xs0 = nc.dram_tensor("xs0", [128, g_, BS], fp16, kind="ExternalInput").ap()
    msk = nc.dram_tensor("msk", [128, 128], fp16, kind="ExternalInput").ap()
    ind = nc.dram_tensor("ind", [128, BS], fp16, kind="ExternalInput").ap()
    vind = nc.dram_tensor("vind", [BS, 128], fp16, kind="ExternalInput").ap()
    if nonzero_b0:
        wp0 = nc.dram_tensor("wp0", [g_, 128, F], fp16, kind="ExternalInput").ap()
        b0p = nc.dram_tensor("b0p", [128, g_ * J], f32, kind="ExternalInput").ap()
    v_out = nc.dram_tensor("v_out", [BS, F], f32, kind="ExternalOutput").ap()

    from contextlib import ExitStack

    with tile.TileContext(nc) as tc:
        for _body in range(n_bodies):
            _sfx = "" if _body == 0 else "@%d" % _body
            with ExitStack() as ctx:
                ent = ctx.enter_context
                uhat_pool = ent(tc.tile_pool(name="uhat" + _sfx, bufs=1))
                alog_pool = ent(tc.tile_pool(name="alog" + _sfx, bufs=1))
                cst_pool = ent(tc.tile_pool(name="cst" + _sfx, bufs=1))
                sm_pool = ent(tc.tile_pool(name="sm" + _sfx, bufs=SM_BUFS))
                vrep_pool = ent(tc.tile_pool(name="vrep" + _sfx, bufs=2))
                sq_pool = ent(tc.tile_pool(name="sq" + _sfx, bufs=1))
                s0_psum = ent(tc.tile_pool(name="s0ps" + _sfx, bufs=1, space="PSUM"))
                phase1 = ExitStack()
                xs0_pool = phase1.enter_context(tc.tile_pool(name="xs0p" + _sfx, bufs=1))
                w_pool = phase1.enter_context(tc.tile_pool(name="wstream" + _sfx, bufs=W_BUFS))
                l_pool = phase1.enter_context(tc.tile_pool(name="lstream" + _sfx, bufs=4))
                mm_psum = phase1.enter_context(
                    tc.tile_pool(name="mmps" + _sfx, bufs=3, space="PSUM")
                )
                # ---- persistent SBUF tensors ----
                uhat = uhat_pool.tile([128, g_ * F], fp16)       # (g,(e,j)) per part
                uv = uhat[:].rearrange("p (g f) -> p g f", g=g_)
                a1 = alog_pool.tile([128, g_ * J], f32)          # iteration-1 logits
                a1v = a1[:].rearrange("p (g j) -> p g j", g=g_)
                xs0_sb = xs0_pool.tile([128, g_ * BS], fp16)
                xs0v = xs0_sb[:].rearrange("p (g b) -> p g b", g=g_)
                ind_sb = cst_pool.tile([128, BS], fp16)
                vind_sb = cst_pool.tile([BS, 128], fp16)
                msk_sb = cst_pool.tile([128, 128], fp16)
                if nonzero_b0:
                    b0_sb = alog_pool.tile([128, g_ * J], f32)
                    b0v = b0_sb[:].rearrange("p (g j) -> p g j", g=g_)

                nc.sync.dma_start(xs0_sb[:], xs0.rearrange("p g b -> p (g b)"))
                nc.sync.dma_start(ind_sb[:], ind)
                nc.sync.dma_start(vind_sb[:], vind)
                nc.sync.dma_start(msk_sb[:], msk)
                if nonzero_b0:
                    nc.sync.dma_start(b0_sb[:], b0p)

                # ---- phase 1: u_hat + S0 ----
                # W DMA in batches of GD groups.  The block-diagonal lhsT is
                # built on-chip from xs0 (= x/J) with a mask multiply, so u_hat
                # lands in PSUM scaled by 1/J and the psum->SBUF copy multiplies
                # by J.  Copies alternate between ACT and DVE.
                GD = 8
                mulJ = float(J)
                s0 = s0_psum.tile([BS, F], f32)
                assert g_ % GD == 0
                _mm = mybir.AluOpType.mult
                for gd in range(g_ // GD):
                    g0 = gd * GD
                    wt = w_pool.tile([128, GD * F], fp16)
                    wtv = wt[:].rearrange("p (g f) -> p g f", g=GD)
                    nc.sync.dma_start(wtv, wp[g0:g0 + GD].rearrange("g p f -> p g f"))
                    if nonzero_b0:
                        w0t = w_pool.tile([128, GD * F], fp16, tag="w0t")
                        w0tv = w0t[:].rearrange("p (g f) -> p g f", g=GD)
                        nc.sync.dma_start(
                            w0tv, wp0[g0:g0 + GD].rearrange("g p f -> p g f")
                        )
                    for h in range(GD // 2):
                        ps = mm_psum.tile([128, 2 * F], f32)
                        for k in range(2):
                            g = g0 + h * 2 + k
                            lt = l_pool.tile([128, 128], fp16)
                            xsb = xs0v[:, g][:, None, :].broadcast_to([128, IL, BS])
                            nc.vector.tensor_tensor(
                                lt[:].rearrange("p (i b) -> p i b", i=IL),
                                xsb, msk_sb[:].rearrange("p (i b) -> p i b", i=IL),
                                op=_mm,
                            )
                            nc.tensor.matmul(
                                ps[:, k * F:(k + 1) * F], lhsT=lt[:],
                                rhs=wtv[:, h * 2 + k], start=True, stop=True,
                            )
                            s0_rhs = w0tv[:, h * 2 + k] if nonzero_b0 else wtv[:, h * 2 + k]
                            nc.tensor.matmul(
                                s0[:], lhsT=xs0v[:, g], rhs=s0_rhs,
                                start=(g == 0), stop=(g == g_ - 1),
                            )
                        gg = g0 + h * 2
                        if gg % 4 == 0:
                            nc.scalar.activation(
                                uhat[:, gg * F:(gg + 2) * F], ps[:],
                                mybir.ActivationFunctionType.Copy, scale=mulJ,
                            )
                        else:
                            nc.vector.tensor_scalar_mul(
                                uhat[:, gg * F:(gg + 2) * F], ps[:], mulJ
                            )

                # free the phase-1 streaming pools; routing pools reuse the space
                phase1.close()
                p_pool = ent(tc.tile_pool(name="ptree" + _sfx, bufs=P_BUFS))
                s_psum = ent(tc.tile_pool(name="sps" + _sfx, bufs=2, space="PSUM"))
                vr_psum = ent(tc.tile_pool(name="vrps" + _sfx, bufs=1, space="PSUM"))

                byp = mybir.AluOpType.bypass
                mul = mybir.AluOpType.mult
                add = mybir.AluOpType.add

                def squash(s_ps, out_dt, out_pool):
                    """s_ps: PSUM [BS, F] f32 in (e,j) layout -> V tile [BS, F]."""
                    sqv = sq_pool.tile([BS, F], f32, tag="sqv")
                    nc.scalar.activation(
                        sqv[:], s_ps[:], mybir.ActivationFunctionType.Square
                    )
                    s2 = sq_pool.tile([BS, J], f32, tag="s2")
                    # reduce over e (outer dim): view (j, e) with e innermost
                    sq3 = sqv[:].rearrange("p (e j) -> p j e", e=E)
                    nc.vector.tensor_reduce(
                        s2[:], sq3, axis=mybir.AxisListType.X, op=add
                    )
                    # rt = sqrt(s2 + 1e-7).  Sqrt lives in a different ACT
                    # function set than Exp, but squash runs only at pass
                    # boundaries so the table reload cost is paid ~6 times total.
                    rt = sq_pool.tile([BS, J], f32, tag="rt")
                    nc.scalar.activation(
                        rt[:], s2[:], mybir.ActivationFunctionType.Sqrt, bias=1e-7
                    )
                    den = sq_pool.tile([BS, J], f32, tag="den")
                    nc.vector.scalar_tensor_tensor(
                        den[:], s2[:], 1.0, rt[:], op0=add, op1=mul
                    )
                    rden = sq_pool.tile([BS, J], f32, tag="rden")
                    nc.vector.reciprocal(rden[:], den[:])
                    sc = sq_pool.tile([BS, J], f32, tag="sc")
                    nc.vector.tensor_tensor(sc[:], s2[:], rden[:], op=mul)
                    # V = S * sc (broadcast sc over e)
                    vt = out_pool.tile([BS, F], out_dt, tag="vtile")
                    scb = sc[:][:, None, :].broadcast_to([BS, E, J])
                    nc.vector.scalar_tensor_tensor(
                        vt[:].rearrange("p (e j) -> p e j", e=E),
                        s_ps[:].rearrange("p (e j) -> p e j", e=E),
                        0.0, scb, op0=byp, op1=mul,
                    )
                    return vt

                def make_vrep(v_sb):
                    """v_sb [BS, F] fp16 -> V replicated to 128 partitions fp16."""
                    vr_ps = vr_psum.tile([128, F], f32)
                    nc.tensor.matmul(
                        vr_ps[:], lhsT=vind_sb[:], rhs=v_sb[:], start=True, stop=True
                    )
                    vr = vrep_pool.tile([128, F], fp16)
                    nc.scalar.activation(
                        vr[:], vr_ps[:], mybir.ActivationFunctionType.Copy
                    )
                    return vr

                n_mac = g_ // GB
                exp_f = mybir.ActivationFunctionType.Exp
                ln_f = mybir.ActivationFunctionType.Ln

                def routing_pass(vr, it, s_ps):
                    """One routing iteration: logits update, softmax, S matmul.

                    All large DVE ops are TensorTensor (2x mode for packed fp16).
                    The e-reduction tree runs in place inside the product tile.
                    Two-stage software pipeline: stage A (P, tree, a, exp) of
                    macro m+1 is emitted before stage B (sumexp, c, T, S-matmuls)
                    of macro m so DVE never stalls on the ACT exp.
                    """
                    def stage_a(m):
                        g0 = m * GB
                        u8 = uv[:, g0:g0 + GB]                       # [128, GB, F]
                        u8e = u8.rearrange("p g (e j) -> p g e j", e=E)
                        # P = u_hat * V_rep  (TT, 2x)
                        p8 = p_pool.tile([128, GB * F], fp16)
                        p8v = p8[:].rearrange("p (g e j) -> p g e j", g=GB, e=E)
                        vrb = vr[:][:, None, :].broadcast_to([128, GB, F]).rearrange(
                            "p g (e j) -> p g e j", e=E
                        )
                        _peng = (
                            nc.gpsimd
                            if GP_EVERY and m % GP_EVERY == GP_EVERY - 1
                            else nc.vector
                        )
                        _peng.tensor_tensor(p8v, u8e, vrb, op=mul)
                        # e-reduction tree 16->8->4->2->1, in place in p8
                        nc.vector.tensor_tensor(
                            p8v[:, :, 0:8], p8v[:, :, 0:8], p8v[:, :, 8:16], op=add
                        )
                        nc.vector.tensor_tensor(
                            p8v[:, :, 0:4], p8v[:, :, 0:4], p8v[:, :, 4:8], op=add
                        )
                        nc.vector.tensor_tensor(
                            p8v[:, :, 0:2], p8v[:, :, 0:2], p8v[:, :, 2:4], op=add
                        )
                        # logits
                        if it == 1:
                            lg4v = a1v[:, g0:g0 + GB]                # write a1 in place
                            nc.vector.tensor_tensor(
                                lg4v, p8v[:, :, 0], p8v[:, :, 1], op=add
                            )
                            if nonzero_b0:
                                nc.vector.tensor_tensor(
                                    lg4v, lg4v, b0v[:, g0:g0 + GB], op=add
                                )
                        else:
                            a2 = sm_pool.tile([128, GB * J], fp16, tag="a2")
                            a2v = a2[:].rearrange("p (g j) -> p g j", g=GB)
                            nc.vector.tensor_tensor(
                                a2v, p8v[:, :, 0], p8v[:, :, 1], op=add
                            )
                            lg = sm_pool.tile([128, GB * J], f32, tag="lg")
                            lg4v = lg[:].rearrange("p (g j) -> p g j", g=GB)
                            nc.vector.tensor_tensor(
                                lg4v, a2v, a1v[:, g0:g0 + GB], op=add
                            )
                        # softmax over j, without max-subtraction: logits are
                        # bounded (|b| < ~25 for this distribution), so f32 exp is
                        # safe, and one batched ACT exp covers all GB groups.
                        ex = sm_pool.tile([128, GB * J], f32, tag="ex")
                        exv = ex[:].rearrange("p (g j) -> p g j", g=GB)
                        se = sm_pool.tile([128, GB], f32, tag="se")
                        for k in range(GB):
                            nc.scalar.activation(
                                exv[:, k], lg4v[:, k], exp_f,
                                accum_out=se[:][:, k:k + 1],
                            )
                        return p8, u8e, ex, se

                    def stage_b(m, p8, u8e, ex, se):
                        g0 = m * GB
                        p8v = p8[:].rearrange("p (g e j) -> p g e j", g=GB, e=E)
                        exv = ex[:].rearrange("p (g j) -> p g j", g=GB)
                        rc = sm_pool.tile([128, GB], f32, tag="rc")
                        nc.vector.reciprocal(rc[:], se[:])
                        # c = exp * (1/sumexp), broadcast rc over j
                        cc = sm_pool.tile([128, GB * J], fp16, tag="cc")
                        ccv = cc[:].rearrange("p (g j) -> p g j", g=GB)
                        rcb = rc[:][:, :, None].broadcast_to([128, GB, J])
                        nc.vector.tensor_tensor(ccv, exv, rcb, op=mul)
                        # T = u_hat * c (batched over GB groups), written into the
                        # now-dead p8 tile; S-matmul per group reads slices of it
                        ccb = cc[:].rearrange("p (g j) -> p g j", g=GB)[
                            :, :, None, :
                        ].broadcast_to([128, GB, E, J])
                        nc.vector.tensor_tensor(p8v, u8e, ccb, op=mul)
                        for k in range(GB):
                            g = g0 + k
                            nc.tensor.matmul(
                                s_ps[:], lhsT=ind_sb[:],
                                rhs=p8[:, k * F:(k + 1) * F],
                                start=(g == 0), stop=(g == g_ - 1),
                            )

                    prev = None
                    for m in range(n_mac):
                        cur = (m, *stage_a(m))
                        if prev is not None:
                            stage_b(*prev)
                        prev = cur
                    stage_b(*prev)

                # ---- routing ----
                if n_passes == 0:
                    vfin = squash(s0, f32, sq_pool)
                elif n_passes == 1:
                    v0 = squash(s0, fp16, sq_pool)
                    vr0 = make_vrep(v0)
                    s1 = s_psum.tile([BS, F], f32, tag="spsum")
                    routing_pass(vr0, 1, s1)
                    vfin = squash(s1, f32, sq_pool)
                else:
                    v0 = squash(s0, fp16, sq_pool)
                    vr0 = make_vrep(v0)
                    s1 = s_psum.tile([BS, F], f32, tag="spsum")
                    routing_pass(vr0, 1, s1)
                    v1 = squash(s1, fp16, sq_pool)
                    vr1 = make_vrep(v1)
                    s2_ps = s_psum.tile([BS, F], f32, tag="spsum")
                    routing_pass(vr1, 2, s2_ps)
                    vfin = squash(s2_ps, f32, sq_pool)
                nc.sync.dma_start(v_out, vfin[:])

    nc.compile()
    return nc


def _prep_inputs(inputs, W, b0, n_groups):
    """Host-side data layout. Returns (in_maps, nonzero_b0)."""
    g_ = n_groups
    i_ = g_ * IL
    nonzero_b0 = bool(np.any(b0[:i_]))

    w = np.ascontiguousarray(W[:i_]).astype(np.float32)
    # [i, j, d, e] -> [g, il, d, e, j] -> [g, 128, 512]
    wp = (
        w.reshape(g_, IL, J, D, E)
        .transpose(0, 1, 3, 4, 2)
        .reshape(g_, 128, J * E)
        .astype(np.float16)
    )

    shared = {"wp": wp}
    if nonzero_b0:
        c0 = b0[:i_].astype(np.float64)
        c0 = np.exp(c0 - c0.max(axis=1, keepdims=True))
        c0 = (c0 / c0.sum(axis=1, keepdims=True)).astype(np.float32)  # [i, J]
        # the on-chip S0 matmul uses xs0 = x/J as lhsT, so scale by J here
        w0 = w.reshape(g_, IL, J, D, E) * (c0 * J).reshape(g_, IL, J, 1, 1)
        wp0 = (
            w0.transpose(0, 1, 3, 4, 2).reshape(g_, 128, J * E).astype(np.float16)
        )
        shared["wp0"] = wp0
        # row-wise max-shift keeps the on-chip exp (no max subtraction) safe
        b0s = b0[:i_] - b0[:i_].max(axis=1, keepdims=True)
        b0p = np.broadcast_to(
            b0s.reshape(g_, IL, 1, J), (g_, IL, BS, J)
        )  # [g, il, b, j] ; partition = il*8+b
        shared["b0p"] = (
            np.ascontiguousarray(b0p.transpose(1, 2, 0, 3))
            .reshape(128, g_ * J)
            .astype(np.float32)
        )

    eye = np.eye(BS, dtype=np.float16)
    shared["ind"] = np.tile(eye, (IL, 1))          # [128, 8]
    shared["vind"] = shared["ind"].T.copy()        # [8, 128]

    shared["msk"] = np.kron(
        np.eye(IL, dtype=np.float16), np.ones((D, BS), np.float16)
    )  # [128, 128], 1 where il == il2

    in_maps = []
    for c in range(NC_CORES):
        xc = inputs[c * BS:(c + 1) * BS, :i_].astype(np.float32)  # [8, i, d]
        xt = xc.reshape(BS, g_, IL, D).transpose(1, 2, 3, 0)      # [g, il, d, b]
        xs0 = (
            np.ascontiguousarray(xt.transpose(1, 2, 0, 3)).reshape(128, g_ * BS)
            / np.float32(J)
        ).astype(np.float16).reshape(128, g_, BS)
        in_maps.append(dict(shared, xs0=xs0))
    return in_maps, nonzero_b0


def _get_program(n_groups, nonzero_b0):
    key = (n_groups, nonzero_b0)
    if key not in _CACHE:
        _CACHE[key] = _build_program(n_groups, nonzero_b0)
    return _CACHE[key]


def run_on_hw(inputs, W, b0, n_groups=G, trace=False):
    from concourse.bass_utils import run_bass_kernel_spmd

    in_maps, nonzero_b0 = _prep_inputs(inputs, W, b0, n_groups)
    nc = _get_program(n_groups, nonzero_b0)
    res = run_bass_kernel_spmd(nc, in_maps, list(range(NC_CORES)), trace=trace)
    outs = []
    for c in range(NC_CORES):
        v = res.results[c]["v_out"]                # [BS, 512] f32, (e,j) layout
        outs.append(v.reshape(BS, E, J).transpose(0, 2, 1))  # [BS, J, E]
    return np.concatenate(outs, axis=0).astype(np.float32), res


def kernel(inputs, W, b0):
    inputs = np.asarray(inputs, dtype=np.float32)
    W = np.asarray(W, dtype=np.float32)
    b0 = np.asarray(b0, dtype=np.float32)
    out, _ = run_on_hw(inputs, W, b0)
    return out



# revision 4
# speedup vs baseline: 1.3811x; 1.3811x over previous
"""CapsuleLayer dynamic-routing kernel for 8 Trainium2 NeuronCores.

Problem: u_hat[b,i,j,e] = einsum('bid,ijde->bije', x, W) with
B=64, I=2304, D=8, J=32, E=16, followed by NUM_ROUTING=3 softmax
routing iterations.  Output V = squash(S_2) with shape [B, J, E].

Strategy (data-parallel over batch, 8 b per core):
 - Host pre-lays W into fp16 tiles [G=144, 128, 512] with partition
   p = (i_loc*8 + d) and free f = (e*32 + j); W streams through SBUF once.
 - A block-diagonal lhsT (built on-chip from x with one batched mask
   multiply per 8-group chunk) makes ONE matmul produce u_hat for
   16 i's x 8 b x (all j,e) per group; a second matmul per group
   accumulates J*S_0 = sum_i u_hat in PSUM (the 1/J lands in the squash
   scale).  PSUM->SBUF evacuation is a pure dtype-converting copy,
   round-robined over DVE/ACT/Pool.
 - u_hat stays resident in SBUF as fp16 [128=(il,b), G*512=(g,(e,j))].
 - Routing iterations exploit b-update linearity: logits(it2) =
   b0 + u.(V0+V1), so no logit tensor is carried between iterations.
 - Per iteration the agreement reduction over e runs on the TENSOR
   engine: P = u*V_rep is written in (e,g,j) order and 16 identity-lhsT
   matmuls accumulate the e-slices into a PSUM logit tile, freeing the
   DVE of the old reduction tree.  exp() runs batched on ACT straight
   from PSUM; the softmax denominator is a small j-tree on GPSIMD.
 - DVE keeps only: P = u*V_rep, T = u*c (both fp16 2x TensorTensor),
   c = exp*recip, and the squash tail.  A knob offloads a subset of the
   P/T macro-multiplies to GPSIMD for engine balance.
"""

import sys

import numpy as np

sys.path.insert(0, "/opt/trn_rl_repo")

B, I, D, J, E = 64, 2304, 8, 32, 16
NC_CORES = 8
BS = B // NC_CORES          # 8 batch elements per core
IL = 16                     # i's per group
G = I // IL                 # 144 groups
F = J * E                   # 512 free elements per group
GB = 6                      # groups per batched macro-op
GJ = GB * J                 # free size of one macro logit tile
P_BUFS = 3                  # product-tile buffering
W_BUFS = 3                  # W-stream buffering
SM_BUFS = 3                 # softmax small-tile buffering
A_BUFS = 2                  # logit PSUM buffering
NUM_ROUTING = 3

# engine-balance knobs
POOL_P_EVERY = 0            # every Nth P-multiply goes to GPSIMD (0=off)
POOL_T_EVERY = 0            # every Nth T-multiply goes to GPSIMD (0=off)
Z_ON_POOL = True            # softmax denominator tree on GPSIMD
# PSUM->SBUF evac engine pattern, cycled per 2-group chunk
EVAC_PATTERN = "da"         # d=DVE, a=ACT (GPSIMD cannot access PSUM)

_CACHE = {}


def _build_program(n_groups, nonzero_b0):
    import concourse.bass as bass
    import concourse.mybir as mybir
    import concourse.tile as tile
    from concourse import bacc

    fp16 = mybir.dt.float16
    f32 = mybir.dt.float32

    nc = bacc.Bacc("TRN2", target_bir_lowering=False, debug=False)

    # register the squash-epsilon constant for activation bias
    eps_t = nc.alloc_sbuf_tensor("const-f32-eps", [128, 1], f32)
    nc.gpsimd.memset(eps_t.ap(), 1e-7)
    nc.const_aps.aps[(f32, 1e-7)] = eps_t.ap()
    nc.all_engine_barrier()

    g_ = n_groups
    wp = nc.dram_tensor("wp", [g_, 128, F], fp16, kind="ExternalInput").ap()
    xs = nc.dram_tensor("xs", [128, g_, BS], fp16, kind="ExternalInput").ap()
    msk = nc.dram_tensor("msk", [128, 128], fp16, kind="ExternalInput").ap()
    ind = nc.dram_tensor("ind", [128, BS], fp16, kind="ExternalInput").ap()
    vind = nc.dram_tensor("vind", [BS, 128], fp16, kind="ExternalInput").ap()
    idn = nc.dram_tensor("idn", [128, 128], fp16, kind="ExternalInput").ap()
    if nonzero_b0:
        wp0 = nc.dram_tensor("wp0", [g_, 128, F], fp16, kind="ExternalInput").ap()
        b0p = nc.dram_tensor("b0p", [128, g_ * J], fp16, kind="ExternalInput").ap()
    v_out = nc.dram_tensor("v_out", [BS, F], f32, kind="ExternalOutput").ap()

    from contextlib import ExitStack

    byp = mybir.AluOpType.bypass
    mul = mybir.AluOpType.mult
    add = mybir.AluOpType.add

    with tile.TileContext(nc) as tc:
        with ExitStack() as ctx:
            ent = ctx.enter_context
            uhat_pool = ent(tc.tile_pool(name="uhat", bufs=1))
            cst_pool = ent(tc.tile_pool(name="cst", bufs=1))
            sm_pool = ent(tc.tile_pool(name="sm", bufs=SM_BUFS))
            vrep_pool = ent(tc.tile_pool(name="vrep", bufs=1))
            sq_pool = ent(tc.tile_pool(name="sq", bufs=1))
            s0_psum = ent(tc.tile_pool(name="s0ps", bufs=1, space="PSUM"))
            phase1 = ExitStack()
            xs_pool = phase1.enter_context(tc.tile_pool(name="xsp", bufs=1))
            w_pool = phase1.enter_context(tc.tile_pool(name="wstream", bufs=W_BUFS))
            l_pool = phase1.enter_context(tc.tile_pool(name="lstream", bufs=3))
            mm_psum = phase1.enter_context(
                tc.tile_pool(name="mmps", bufs=3, space="PSUM")
            )
            # ---- persistent SBUF tensors ----
            uhat = uhat_pool.tile([128, g_ * F], fp16)       # (g,(e,j)) per part
            uv = uhat[:].rearrange("p (g f) -> p g f", g=g_)
            xs_sb = xs_pool.tile([128, g_ * BS], fp16)
            xsv = xs_sb[:].rearrange("p (g b) -> p g b", g=g_)
            ind_sb = cst_pool.tile([128, BS], fp16)
            vind_sb = cst_pool.tile([BS, 128], fp16)
            msk_sb = cst_pool.tile([128, 128], fp16)
            idn_sb = cst_pool.tile([128, 128], fp16)
            if nonzero_b0:
                b0_sb = cst_pool.tile([128, g_ * J], fp16)
                b0v = b0_sb[:].rearrange("p (g j) -> p g j", g=g_)

            nc.sync.dma_start(xs_sb[:], xs.rearrange("p g b -> p (g b)"))
            nc.sync.dma_start(ind_sb[:], ind)
            nc.sync.dma_start(vind_sb[:], vind)
            nc.sync.dma_start(msk_sb[:], msk)
            nc.sync.dma_start(idn_sb[:], idn)
            if nonzero_b0:
                nc.sync.dma_start(b0_sb[:], b0p)

            # ---- phase 1: u_hat + J*S0 ----
            # W DMA in batches of GD groups.  One batched mask multiply
            # builds the block-diagonal lhsT for all GD groups; u_hat
            # lands in PSUM unscaled and is evacuated by pure copies
            # round-robined over DVE/ACT/Pool.
            GD = 8
            s0 = s0_psum.tile([BS, F], f32)
            assert g_ % GD == 0
            evac_idx = 0
            for gd in range(g_ // GD):
                g0 = gd * GD
                wt = w_pool.tile([128, GD * F], fp16)
                wtv = wt[:].rearrange("p (g f) -> p g f", g=GD)
                nc.sync.dma_start(wtv, wp[g0:g0 + GD].rearrange("g p f -> p g f"))
                if nonzero_b0:
                    w0t = w_pool.tile([128, GD * F], fp16, tag="w0t")
                    w0tv = w0t[:].rearrange("p (g f) -> p g f", g=GD)
                    nc.sync.dma_start(
                        w0tv, wp0[g0:g0 + GD].rearrange("g p f -> p g f")
                    )
                # batched block-diagonal lhsT for GD groups: one DVE op
                lt = l_pool.tile([128, GD * 128], fp16)
                ltv = lt[:].rearrange("p (g i b) -> p g i b", g=GD, i=IL)
                xsb = xsv[:, g0:g0 + GD][:, :, None, :].broadcast_to(
                    [128, GD, IL, BS]
                )
                mskb = msk_sb[:].rearrange("p (i b) -> p i b", i=IL)[
                    :, None, :, :
                ].broadcast_to([128, GD, IL, BS])
                nc.vector.tensor_tensor(ltv, xsb, mskb, op=mul)
                for h in range(GD // 2):
                    ps = mm_psum.tile([128, 2 * F], f32)
                    for k in range(2):
                        g = g0 + h * 2 + k
                        nc.tensor.matmul(
                            ps[:, k * F:(k + 1) * F],
                            lhsT=lt[:, (h * 2 + k) * 128:(h * 2 + k + 1) * 128],
                            rhs=wtv[:, h * 2 + k], start=True, stop=True,
                        )
                        s0_rhs = w0tv[:, h * 2 + k] if nonzero_b0 else wtv[:, h * 2 + k]
                        nc.tensor.matmul(
                            s0[:], lhsT=xsv[:, g], rhs=s0_rhs,
                            start=(g == 0), stop=(g == g_ - 1),
                        )
                    gg = g0 + h * 2
                    dst = uhat[:, gg * F:(gg + 2) * F]
                    e = EVAC_PATTERN[evac_idx % len(EVAC_PATTERN)]
                    evac_idx += 1
                    if e == "a":
                        nc.scalar.activation(
                            dst, ps[:], mybir.ActivationFunctionType.Copy
                        )
                    elif e == "p":
                        nc.gpsimd.tensor_copy(dst, ps[:])
                    else:
                        nc.vector.tensor_copy(dst, ps[:])

            # free the phase-1 streaming pools; routing pools reuse the space
            phase1.close()
            p_pool = ent(tc.tile_pool(name="ptree", bufs=P_BUFS))
            s_psum = ent(tc.tile_pool(name="sps", bufs=2, space="PSUM"))
            a_psum = ent(tc.tile_pool(name="aps", bufs=A_BUFS, space="PSUM"))
            vr_psum = ent(tc.tile_pool(name="vrps", bufs=1, space="PSUM"))

            def squash(s_ps, out_dt, out_pool, inv_scale):
                """s_ps: PSUM [BS, F] f32 = S/inv_scale -> V tile [BS, F]."""
                sqv = sq_pool.tile([BS, F], f32, tag="sqv")
                nc.scalar.activation(
                    sqv[:], s_ps[:], mybir.ActivationFunctionType.Square,
                    scale=float(inv_scale),
                )
                s2 = sq_pool.tile([BS, J], f32, tag="s2")
                # reduce over e (outer dim): view (j, e) with e innermost
                sq3 = sqv[:].rearrange("p (e j) -> p j e", e=E)
                nc.vector.tensor_reduce(
                    s2[:], sq3, axis=mybir.AxisListType.X, op=add
                )
                rt = sq_pool.tile([BS, J], f32, tag="rt")
                nc.scalar.activation(
                    rt[:], s2[:], mybir.ActivationFunctionType.Sqrt, bias=1e-7
                )
                den = sq_pool.tile([BS, J], f32, tag="den")
                nc.vector.scalar_tensor_tensor(
                    den[:], s2[:], 1.0, rt[:], op0=add, op1=mul
                )
                rden = sq_pool.tile([BS, J], f32, tag="rden")
                nc.vector.reciprocal(rden[:], den[:])
                sc = sq_pool.tile([BS, J], f32, tag="sc")
                nc.vector.tensor_tensor(sc[:], s2[:], rden[:], op=mul)
                # V = (S/beta) * sc (broadcast sc over e)
                vt = out_pool.tile([BS, F], out_dt, tag="vtile")
                scb = sc[:][:, None, :].broadcast_to([BS, E, J])
                nc.vector.scalar_tensor_tensor(
                    vt[:].rearrange("p (e j) -> p e j", e=E),
                    s_ps[:].rearrange("p (e j) -> p e j", e=E),
                    float(inv_scale), scb, op0=mul, op1=mul,
                )
                return vt

            def make_vrep(v_sb):
                """v_sb [BS, F] fp16 -> V replicated to 128 partitions fp16."""
                vr_ps = vr_psum.tile([128, F], f32)
                nc.tensor.matmul(
                    vr_ps[:], lhsT=vind_sb[:], rhs=v_sb[:], start=True, stop=True
                )
                vr = vrep_pool.tile([128, F], fp16, tag="vr%d" % make_vrep.n)
                make_vrep.n += 1
                nc.scalar.activation(
                    vr[:], vr_ps[:], mybir.ActivationFunctionType.Copy
                )
                return vr

            make_vrep.n = 0

            n_mac = g_ // GB
            exp_f = mybir.ActivationFunctionType.Exp
            zeng = nc.gpsimd if Z_ON_POOL else nc.vector

            def routing_pass(vr, it, s_ps):
                """One routing iteration.

                stage A: P = u*V_rep written in (e,g,j) order (DVE TT 2x,
                or GPSIMD for offloaded macros); 16 identity-lhsT matmuls
                accumulate the e-slices into a PSUM logit tile; batched
                f32 exp on ACT; j-sum tree for the denominator on GPSIMD.
                stage B: c = exp*(1/Z) (DVE), T = u*c (TT 2x), and PE
                contracts over i with the indicator lhsT into S.
                Two-stage software pipelining keeps DVE busy across the
                ACT/PE handoffs.
                """
                def stage_a(m):
                    g0 = m * GB
                    u8 = uv[:, g0:g0 + GB]                       # [128, GB, F]
                    u8e = u8.rearrange("p g (e j) -> p e g j", e=E)
                    p8 = p_pool.tile([128, GB * F], fp16, tag="p8")
                    p8v = p8[:].rearrange("p (e g j) -> p e g j", e=E, g=GB)
                    vrb = vr[:].rearrange("p (e j) -> p e j", e=E)[
                        :, :, None, :
                    ].broadcast_to([128, E, GB, J])
                    peng = (
                        nc.gpsimd
                        if POOL_P_EVERY and m % POOL_P_EVERY == POOL_P_EVERY - 1
                        else nc.vector
                    )
                    peng.tensor_tensor(p8v, u8e, vrb, op=mul)
                    # e-reduction on the tensor engine: 16 accumulating
                    # identity matmuls over contiguous e-slices of p8
                    aps = a_psum.tile([128, GJ], f32)
                    for e in range(E):
                        nc.tensor.matmul(
                            aps[:], lhsT=idn_sb[:],
                            rhs=p8[:, e * GJ:(e + 1) * GJ],
                            start=(e == 0),
                            stop=(e == E - 1 and not nonzero_b0),
                        )
                    if nonzero_b0:
                        nc.tensor.matmul(
                            aps[:], lhsT=idn_sb[:],
                            rhs=b0_sb[:, g0 * J:(g0 + GB) * J],
                            start=False, stop=True,
                        )
                    ex = sm_pool.tile([128, GJ], f32, tag="ex")
                    nc.scalar.activation(ex[:], aps[:], exp_f)
                    # denominator: j-tree 32->16->8->4->2->1 into scratch
                    zz = sm_pool.tile([128, GB * 16], f32, tag="zz")
                    zzv = zz[:].rearrange("p (g h) -> p g h", g=GB)
                    exv = ex[:].rearrange("p (g j) -> p g j", g=GB)
                    zeng.tensor_tensor(
                        zzv, exv[:, :, 0:16], exv[:, :, 16:32], op=add
                    )
                    zeng.tensor_tensor(
                        zzv[:, :, 0:8], zzv[:, :, 0:8], zzv[:, :, 8:16], op=add
                    )
                    zeng.tensor_tensor(
                        zzv[:, :, 0:4], zzv[:, :, 0:4], zzv[:, :, 4:8], op=add
                    )
                    zeng.tensor_tensor(
                        zzv[:, :, 0:2], zzv[:, :, 0:2], zzv[:, :, 2:4], op=add
                    )
                    zeng.tensor_tensor(
                        zzv[:, :, 0:1], zzv[:, :, 0:1], zzv[:, :, 1:2], op=add
                    )
                    return p8, u8, ex, zz

                def stage_b(m, p8, u8, ex, zz):
                    g0 = m * GB
                    rc = sm_pool.tile([128, GB], f32, tag="rc")
                    nc.vector.reciprocal(
                        rc[:], zz[:].rearrange("p (g h) -> p g h", g=GB)[:, :, 0]
                    )
                    # c = exp * (1/sumexp), broadcast rc over j; fp16 out
                    cc = sm_pool.tile([128, GJ], fp16, tag="cc")
                    ccv = cc[:].rearrange("p (g j) -> p g j", g=GB)
                    exv = ex[:].rearrange("p (g j) -> p g j", g=GB)
                    rcb = rc[:][:, :, None].broadcast_to([128, GB, J])
                    nc.vector.tensor_tensor(ccv, exv, rcb, op=mul)
                    # T = u * c (broadcast c over e), (g,e,j) order so the
                    # S-matmul rhs slices stay contiguous
                    t8 = p_pool.tile([128, GB * F], fp16, tag="t8")
                    t8v = t8[:].rearrange("p (g e j) -> p g e j", g=GB, e=E)
                    u8e2 = u8.rearrange("p g (e j) -> p g e j", e=E)
                    ccb = cc[:].rearrange("p (g j) -> p g j", g=GB)[
                        :, :, None, :
                    ].broadcast_to([128, GB, E, J])
                    teng = (
                        nc.gpsimd
                        if POOL_T_EVERY and m % POOL_T_EVERY == POOL_T_EVERY - 1
                        else nc.vector
                    )
                    teng.tensor_tensor(t8v, u8e2, ccb, op=mul)
                    for k in range(GB):
                        g = g0 + k
                        nc.tensor.matmul(
                            s_ps[:], lhsT=ind_sb[:],
                            rhs=t8[:, k * F:(k + 1) * F],
                            start=(g == 0), stop=(g == g_ - 1),
                        )

                prev = None
                for m in range(n_mac):
                    cur = (m, *stage_a(m))
                    if prev is not None:
                        stage_b(*prev)
                    prev = cur
                stage_b(*prev)

            # ---- routing (b-linearity: logits(it) = b0 + u.(V0+..+V_{it-1}))
            s0_scale = 1.0 / J if not nonzero_b0 else 1.0
            v0 = squash(s0, fp16, sq_pool, s0_scale)
            vr0 = make_vrep(v0)
            s1 = s_psum.tile([BS, F], f32, tag="spsum")
            routing_pass(vr0, 1, s1)
            v1 = squash(s1, fp16, sq_pool, 1.0)
            vr1 = make_vrep(v1)
            vr01 = vrep_pool.tile([128, F], fp16, tag="vr01")
            nc.vector.tensor_tensor(vr01[:], vr0[:], vr1[:], op=add)
            s2_ps = s_psum.tile([BS, F], f32, tag="spsum")
            routing_pass(vr01, 2, s2_ps)
            vfin = squash(s2_ps, f32, sq_pool, 1.0)
            nc.sync.dma_start(v_out, vfin[:])

    nc.compile()
    return nc


def _prep_inputs(inputs, W, b0, n_groups):
    """Host-side data layout. Returns (in_maps, nonzero_b0)."""
    g_ = n_groups
    i_ = g_ * IL
    nonzero_b0 = bool(np.any(b0[:i_]))

    w = np.ascontiguousarray(W[:i_]).astype(np.float32)
    # [i, j, d, e] -> [g, il, d, e, j] -> [g, 128, 512]
    wp = (
        w.reshape(g_, IL, J, D, E)
        .transpose(0, 1, 3, 4, 2)
        .reshape(g_, 128, J * E)
        .astype(np.float16)
    )

    shared = {"wp": wp}
    if nonzero_b0:
        c0 = b0[:i_].astype(np.float64)
        c0 = np.exp(c0 - c0.max(axis=1, keepdims=True))
        c0 = (c0 / c0.sum(axis=1, keepdims=True)).astype(np.float32)  # [i, J]
        # S0 accumulates x @ (c0-folded W) directly (inv_scale=1)
        w0 = w.reshape(g_, IL, J, D, E) * c0.reshape(g_, IL, J, 1, 1)
        wp0 = (
            w0.transpose(0, 1, 3, 4, 2).reshape(g_, 128, J * E).astype(np.float16)
        )
        shared["wp0"] = wp0
        # logits b0 replicated per (il, b) partition, fp16 for the PE add
        b0p = np.broadcast_to(
            b0[:i_].reshape(g_, IL, 1, J), (g_, IL, BS, J)
        )  # [g, il, b, j] ; partition = il*8+b
        shared["b0p"] = (
            np.ascontiguousarray(b0p.transpose(1, 2, 0, 3))
            .reshape(128, g_ * J)
            .astype(np.float16)
        )

    eye = np.eye(BS, dtype=np.float16)
    shared["ind"] = np.tile(eye, (IL, 1))          # [128, 8]
    shared["vind"] = shared["ind"].T.copy()        # [8, 128]
    shared["idn"] = np.eye(128, dtype=np.float16)  # [128, 128]

    shared["msk"] = np.kron(
        np.eye(IL, dtype=np.float16), np.ones((D, BS), np.float16)
    )  # [128, 128], 1 where il == il2

    in_maps = []
    for c in range(NC_CORES):
        xc = inputs[c * BS:(c + 1) * BS, :i_].astype(np.float32)  # [8, i, d]
        xt = xc.reshape(BS, g_, IL, D).transpose(1, 2, 3, 0)      # [g, il, d, b]
        xsm = (
            np.ascontiguousarray(xt.transpose(1, 2, 0, 3)).reshape(128, g_ * BS)
        ).astype(np.float16).reshape(128, g_, BS)
        in_maps.append(dict(shared, xs=xsm))
    return in_maps, nonzero_b0


def _get_program(n_groups, nonzero_b0):
    key = (n_groups, nonzero_b0)
    if key not in _CACHE:
        _CACHE[key] = _build_program(n_groups, nonzero_b0)
    return _CACHE[key]


def run_on_hw(inputs, W, b0, n_groups=G, trace=False):
    from concourse.bass_utils import run_bass_kernel_spmd

    in_maps, nonzero_b0 = _prep_inputs(inputs, W, b0, n_groups)
    nc = _get_program(n_groups, nonzero_b0)
    res = run_bass_kernel_spmd(nc, in_maps, list(range(NC_CORES)), trace=trace)
    outs = []
    for c in range(NC_CORES):
        v = res.results[c]["v_out"]                # [BS, 512] f32, (e,j) layout
        outs.append(v.reshape(BS, E, J).transpose(0, 2, 1))  # [BS, J, E]
    return np.concatenate(outs, axis=0).astype(np.float32), res


def kernel(inputs, W, b0):
    inputs = np.asarray(inputs, dtype=np.float32)
    W = np.asarray(W, dtype=np.float32)
    b0 = np.asarray(b0, dtype=np.float32)
    out, _ = run_on_hw(inputs, W, b0)
    return out
